# revision 9
# baseline (speedup 1.0000x reference)
"""Multi-head causal attention (B=2, S=2048, D=1024, H=16) on 8 Trainium2 cores.

Sharding: tensor-parallel over heads. Core c computes QKV projection, causal
attention and softmax for heads {2c, 2c+1} over both batches, then an AllToAll
redistributes the attention output so core c owns rows [512c, 512c+512) of the
flattened (B*S, D) activation; each core applies the full output projection to
its row slice. Host code only slices/transposes inputs and concatenates the
per-core output slices.

All matmuls run in bf16 with fp32 PSUM accumulation. The pipeline works in
transposed layout ([dim, seq]) so that softmax reduces over the PSUM partition
axis via a ones-column folded into the PV matmul, and the attention output
lands directly in the layout the output projection consumes.

v2 notes:
- PSUM plan: tag "big" ([128,1024], 2 slots) + tags "o0"/"o1" ([128,512],
  2 slots each) = 8 banks. Everything (QKV chains, score pairs, PV
  accumulators, out-proj chains) is double-buffered, so no phase ever stalls
  the PE on a single-buffered drain (the v1 ps_o drain cost ~5us/query-block
  and re-throttled the PE clock each time).
- Causal masking is applied in PSUM via matmul: identity.T @ staircase adds
  -1e30 to masked score entries before the exp, so exp is always one full
  [128,1024] tile and there are no DVE mask-multiplies / gpsimd memsets.
- Softmax normalization uses reciprocal_approx_fast (~5x faster than the
  iterative-divide reciprocal which cost 3.3us per call on one partition).
- QKV bias+downcast runs on the (otherwise idle) scalar engine.
"""
import numpy as np
from contextlib import ExitStack

import jax
import ml_dtypes

import concourse.bass as bass
import concourse.tile as tile
from concourse import bacc, mybir
from concourse.bass2jax import (
    _bass_exec_p,
    install_neuronx_cc_hook,
    partition_id_tensor,
)
from jax.sharding import Mesh, PartitionSpec
from jax.experimental.shard_map import shard_map

B, S, D, H = 2, 2048, 1024, 16
DH = D // H            # 64
NCORES = 8
HPC = H // NCORES      # heads per core = 2
HD = HPC * DH          # head dims per core = 128
R = B * S              # flattened rows = 4096
RPC = R // NCORES      # rows per core after AllToAll = 512
QB = 512               # query block (also the AllToAll shard size)
KB = 128               # key block
NQB = S // QB          # 4 query blocks per batch
NKB = S // KB          # 16 key blocks per batch
CCH = D // 128         # contraction chunks for D-wide matmuls = 8

BF16 = mybir.dt.bfloat16
F32 = mybir.dt.float32
AF = mybir.ActivationFunctionType
ALU = mybir.AluOpType
NEG = -1e30            # added to masked score entries before exp


def _build(causal: bool, repeat: int = 1, loop_n: int = 0,
           a2a_local: bool = False, parts: str = "full"):
    """Emit the SPMD Bass program (identical on all 8 cores).

    loop_n > 0 builds a timing variant: the whole per-iteration body runs
    inside a hardware For_i loop and the AllToAll is replaced by a local DMA
    copy (collectives cannot sit inside control flow), with the real output
    replaced by a tiny dummy (so the timing loop's donated output buffers are
    negligible to transfer). Used only to measure per-iteration device time.
    """
    timing = loop_n > 0
    nc = bacc.Bacc("TRN2", target_bir_lowering=False, debug=False,
                   num_devices=NCORES)

    xt = nc.dram_tensor("xt", [D, R], BF16, kind="ExternalInput").ap()
    wit = nc.dram_tensor("wit", [D, 3 * HD], BF16, kind="ExternalInput").ap()
    bi_s = nc.dram_tensor("bi_s", [3 * HD], F32, kind="ExternalInput").ap()
    wot = nc.dram_tensor("wot", [D, D], BF16, kind="ExternalInput").ap()
    bo_f = nc.dram_tensor("bo_f", [D], F32, kind="ExternalInput").ap()
    masks = nc.dram_tensor("masks", [KB, KB], BF16, kind="ExternalInput").ap()
    if timing:
        out_t = nc.dram_tensor("out_scratch", [D, RPC], F32).ap()
        dummy = nc.dram_tensor("tiny_out", [1, 16], F32, kind="ExternalOutput").ap()
    else:
        out_t = nc.dram_tensor("out_t", [D, RPC], F32, kind="ExternalOutput").ap()

    with tile.TileContext(nc) as tc, ExitStack() as octx:
        persist = octx.enter_context(tc.tile_pool(name="persist", bufs=1))
        dram = octx.enter_context(tc.tile_pool(name="dram", bufs=1, space="DRAM"))

        # ---- persistent SBUF state (x chunks queued right after wit: the
        # QKV matmuls need them first; wot/bo only matter at the end) ----
        wit_sb = persist.tile([128, CCH, 3 * HD], BF16)
        nc.sync.dma_start(wit_sb[:], wit.rearrange("(cc p) n -> p cc n", p=128))
        bias_sb = persist.tile([128, 3], F32)
        nc.sync.dma_start(bias_sb[:], bi_s.rearrange("(t p) -> p t", p=128))
        xt_pool = octx.enter_context(tc.tile_pool(name="xt_pool", bufs=1))
        xt_sb = xt_pool.tile([128, CCH, R], BF16)
        xt_r = xt.rearrange("(cc p) r -> p cc r", p=128)
        for cc in range(CCH):
            nc.sync.dma_start(xt_sb[:, cc, :], xt_r[:, cc, :])
        wot_sb = persist.tile([128, CCH, D], BF16)
        nc.sync.dma_start(wot_sb[:], wot.rearrange("(cc p) o -> p cc o", p=128))
        bo_sb = persist.tile([128, CCH], F32)
        nc.sync.dma_start(bo_sb[:], bo_f.rearrange("(oc p) -> p oc", p=128))
        # additive triangle mask [128, 128]: NEG strictly below the diagonal
        # in [k, q] layout. identity.T @ tri adds NEG to the masked entries of
        # a diagonal 128x128 score sub-block before the exp.
        tri_sb = persist.tile([128, KB], BF16)
        if causal:
            nc.sync.dma_start(tri_sb[:], masks[:])

        identity = persist.tile([128, 128], BF16)
        from concourse.masks import make_identity
        make_identity(nc, identity[:])

        # qT/kT: [head-dims (2 heads x 64), S] per batch; v: [k rows, 65] blocks
        qt_sb = [persist.tile([128, S], BF16, name=f"qt{b}") for b in range(B)]
        kt_sb = [persist.tile([128, S], BF16, name=f"kt{b}") for b in range(B)]
        # v_sb[h][:, g, :]: col 0 = 1.0, cols 1..31 = 0, cols 32..95 = v rows
        # for global k-block g. The ones column puts the PV denominator on
        # PSUM partition 0 (aligned for the custom-DVE fast reciprocal); the
        # zero pad puts the output dims at partitions 32..95 (PSUM reads must
        # be 32-partition aligned).
        VP, VD0 = 128, 64
        v_sb = [persist.tile([128, B * NKB, VP], BF16, name=f"v{h}")
                for h in range(HPC)]
        for h in range(HPC):
            nc.vector.memset(v_sb[h][:, :, 0:VD0], 0.0)
            nc.vector.memset(v_sb[h][:, :, 0:1], 1.0)

        a2a_in = dram.tile([NCORES, HD, RPC], BF16)
        a2a_out = dram.tile([NCORES, HD, RPC], BF16)
        ao_sb = persist.tile([128, NCORES, RPC], BF16, name="ao_sb")

        # one PSUM pool; tags (all bufs=2, 8 banks total):
        #   big: [128,1024] x2 slots (4 banks)
        #   o0/o1: [128,512] x2 slots each (2+2 banks)
        psum = octx.enter_context(tc.tile_pool(name="psum", bufs=1,
                                               space="PSUM"))
        work = octx.enter_context(tc.tile_pool(name="work", bufs=3))
        epool = octx.enter_context(tc.tile_pool(name="epool", bufs=4))

        def big_ps(name):
            return psum.tile([128, 2 * QB], F32, tag="big", bufs=2, name=name)

        def o_ps(i, name, shape=None, dtype=F32):
            return psum.tile(shape or [128, QB], dtype, tag=f"o{i % 2}",
                             bufs=2, name=name)

        def emit_body(a2a_local: bool):
            # ================= QKV projection (transposed) =================
            for b in range(B):
                for tsr in range(3):  # 0=q, 1=k, 2=v
                    vt_full = None
                    if tsr == 2:
                        vt_full = work.tile([128, S], BF16, tag="vt",
                                            name=f"vt{b}")
                    # 4 concurrent 512-wide chains: big slot (2) + o0 + o1,
                    # double-buffered across passes so the previous pass's
                    # bias/downcast drain never stalls the PE.
                    ps_big = big_ps("ps_qkv")
                    ps_oo = [o_ps(i, "ps_qkvo") for i in range(2)]

                    def chain(rc):
                        if rc < 2:
                            return ps_big[:, rc * QB:(rc + 1) * QB]
                        return ps_oo[rc - 2][:]

                    for cc in range(CCH):
                        for rc in range(S // QB):
                            r0 = b * S + rc * QB
                            nc.tensor.matmul(
                                chain(rc),
                                wit_sb[:, cc, tsr * HD:(tsr + 1) * HD],
                                xt_sb[:, cc, r0:r0 + QB],
                                start=(cc == 0), stop=(cc == CCH - 1),
                            )
                    dst = (qt_sb[b] if tsr == 0 else
                           kt_sb[b] if tsr == 1 else vt_full)
                    # fused per-partition bias add + bf16 downcast, split
                    # across the scalar + vector engines so the pass-boundary
                    # drain latency is halved
                    nc.scalar.activation(dst[:, 0:2 * QB], ps_big[:],
                                         AF.Identity, bias=bias_sb[:, tsr:tsr + 1])
                    for i in range(2):
                        nc.vector.tensor_scalar(
                            dst[:, (2 + i) * QB:(3 + i) * QB], ps_oo[i][:],
                            bias_sb[:, tsr:tsr + 1], None, ALU.add)
                    if tsr == 2:
                        # transpose vT -> v blocks [k rows, dims] on the DMA
                        # crossbar (frees the PE transposes + DVE copies)
                        for h in range(HPC):
                            nc.sync.dma_start_transpose(
                                v_sb[h][:, b * NKB:(b + 1) * NKB,
                                        VD0:VD0 + DH],
                                vt_full[h * DH:(h + 1) * DH, :])

            # ======================= attention =============================
            if parts == "qkv":
                return
            # scores run one k-block ahead of PV so PE never waits on exp.
            # qb descending: the longest query block's dense matmul stretch
            # lands right after QKV, keeping the PE activity gate warm.
            for b in range(B):
                for qb in reversed(range(NQB)):
                    nkb = 4 * (qb + 1) if causal else NKB
                    q0 = qb * QB
                    ps_o = [o_ps(h, f"ps_o{h}", shape=[VP, QB])
                            for h in range(HPC)]

                    def diag_i(kb):
                        # index of kb within this query block's diagonal
                        # 512x512 region, or -1 if kb is fully unmasked
                        i = kb - 4 * qb
                        return i if (causal and 0 <= i < 4) else -1

                    def scores_pair(p):
                        """Two k-blocks (2p, 2p+1) -> one [128,1024] psum per
                        head. For diagonal k-blocks, identity.T @ tri adds NEG
                        to the 128x128 triangle in PSUM (so exp -> 0 there)
                        and the fully-masked query-column prefix is simply
                        skipped by the exp and the PV matmul (N-sliced). MMs
                        alternate heads so consecutive matmuls land on
                        different PE row-groups and overlap in the array."""
                        pss = [big_ps(f"ps_s{h}") for h in range(HPC)]
                        for half in range(2):
                            for h in range(HPC):
                                kb = 2 * p + half
                                nc.tensor.matmul(
                                    pss[h][:, half * QB:(half + 1) * QB],
                                    kt_sb[b][h * DH:(h + 1) * DH,
                                             kb * KB:(kb + 1) * KB],
                                    qt_sb[b][h * DH:(h + 1) * DH, q0:q0 + QB],
                                    start=True, stop=True,
                                )
                        if causal and "nodiag" not in parts:
                            for h in range(HPC):
                                for half in range(2):
                                    i = diag_i(2 * p + half)
                                    if i >= 0:
                                        c = half * QB + i * KB
                                        nc.tensor.matmul(
                                            pss[h][:, c:c + KB],
                                            identity[:], tri_sb[:],
                                            start=False, stop=True,
                                        )
                        # exp skips the fully-masked prefix of half 0 (the
                        # garbage it leaves in half 1's prefix is never read:
                        # the PV rhs is sliced identically)
                        i0 = diag_i(2 * p)
                        c0 = i0 * KB if (i0 > 0 and "nodiag" not in parts) \
                            else 0
                        es = []
                        for h in range(HPC):
                            e = epool.tile([128, 2 * QB], BF16, tag="expT",
                                           name="expT")
                            if "peonly" in parts:
                                es.append(None)
                                continue
                            nc.scalar.activation(e[:, c0:], pss[h][:, c0:],
                                                 AF.Exp, scale=1.0 / 8.0)
                            es.append(e)
                        return es

                    def pv_pair(p, es):
                        for h in range(HPC):
                            for half in range(2):
                                kb = 2 * p + half
                                i = diag_i(kb)
                                cs = i * KB if (i > 0 and
                                                "nodiag" not in parts) else 0
                                rhs = (qt_sb[b][:, q0 + cs:q0 + QB]
                                       if es[h] is None else
                                       es[h][:, half * QB + cs:
                                             (half + 1) * QB])
                                nc.tensor.matmul(
                                    ps_o[h][:, cs:],
                                    v_sb[h][:, b * NKB + kb, :],
                                    rhs,
                                    start=(kb == 0), stop=(kb == nkb - 1),
                                )

                    npair = nkb // 2
                    es_prev = scores_pair(0)
                    for p in range(1, npair):
                        es = scores_pair(p)
                        pv_pair(p - 1, es_prev)
                        es_prev = es
                    pv_pair(npair - 1, es_prev)

                    at = work.tile([128, QB], BF16, tag="attnT", name="attnT")
                    if "nonorm" in parts:
                        nc.vector.tensor_copy(at[0:DH, :],
                                              ps_o[0][VD0:VD0 + DH, :])
                        nc.vector.tensor_copy(at[DH:2 * DH, :],
                                              ps_o[1][VD0:VD0 + DH, :])
                    else:
                        # normalize: fast approximate reciprocal of the
                        # denominators (ones-column row 0 of each PV psum,
                        # partition-aligned for the custom-DVE op),
                        # replicated to all partitions on GPSIMD
                        rc = work.tile([1, 2 * QB], F32, tag="rc", name="rc")
                        for h in range(HPC):
                            nc.vector.reciprocal_approx_fast(
                                rc[0:1, h * QB:(h + 1) * QB],
                                ps_o[h][0:1, :])
                        rpb = work.tile([128, 2 * QB], F32, tag="rpb",
                                        name="rpb")
                        nc.gpsimd.partition_broadcast(rpb[:], rc[0:1, :])
                        for h in range(HPC):
                            nc.vector.tensor_mul(
                                at[h * DH:(h + 1) * DH, :],
                                ps_o[h][VD0:VD0 + DH, :],
                                rpb[h * DH:(h + 1) * DH,
                                    h * QB:(h + 1) * QB])
                    j = b * NQB + qb
                    nc.sync.dma_start(a2a_in[j], at[:])
                    if a2a_local:
                        # timing stand-in for the exchange, overlapped with
                        # the rest of attention (collectives can't sit in
                        # control flow)
                        nc.sync.dma_start(a2a_out[j], a2a_in[j])
                        nc.sync.dma_start(ao_sb[:, j, :], a2a_out[j])

            # ================= AllToAll + output projection ================
            if parts == "qkv+att":
                return
            if not a2a_local:
                nc.gpsimd.collective_compute(
                    "AllToAll", ALU.bypass,
                    replica_groups=[list(range(NCORES))],
                    ins=[a2a_in[:]], outs=[a2a_out[:]],
                )
                for j in range(NCORES):
                    nc.sync.dma_start(ao_sb[:, j, :], a2a_out[j])
            # all 8 output chains live at once across the 8 PSUM banks,
            # contraction chunk j outermost: 56 of 64 matmuls can run before
            # the last exchanged shard lands
            pair_ps = [big_ps(f"ps_outp{i}") for i in range(2)]
            single_ps = [o_ps(i, f"ps_outs{i}", shape=[128, RPC])
                         for i in range(4)]

            def ob_slice(ob):
                if ob < 4:
                    return pair_ps[ob // 2][:, (ob % 2) * RPC:
                                            (ob % 2 + 1) * RPC]
                return single_ps[ob - 4][:]

            for j in range(NCORES):
                for ob in range(CCH):
                    nc.tensor.matmul(
                        ob_slice(ob),
                        wot_sb[:, j, ob * 128:(ob + 1) * 128],
                        ao_sb[:, j, :],
                        start=(j == 0), stop=(j == NCORES - 1),
                    )
            for ob in range(CCH):
                os = work.tile([128, RPC], F32, tag="os", name="os")
                # split the bias-add drain across scalar + vector engines
                if ob % 2:
                    nc.scalar.activation(os[:], ob_slice(ob), AF.Identity,
                                         bias=bo_sb[:, ob:ob + 1])
                else:
                    nc.vector.tensor_scalar(os[:], ob_slice(ob),
                                            bo_sb[:, ob:ob + 1], None, ALU.add)
                nc.sync.dma_start(out_t[ob * 128:(ob + 1) * 128, :], os[:])

        if loop_n:
            with tc.For_i(0, loop_n, 1,
                          hint_engines=(mybir.EngineType.PE,
                                        mybir.EngineType.DVE,
                                        mybir.EngineType.Activation)):
                emit_body(a2a_local=True)
            dsb = persist.tile([1, 16], F32)
            nc.vector.memset(dsb[:], 0.0)
            nc.sync.dma_start(dummy[:], dsb[:])
        else:
            for _ in range(repeat):
                emit_body(a2a_local=a2a_local)

    nc.compile()
    return nc


def _build_a2a_bench(k: int):
    """k back-to-back AllToAlls on the kernel's exchange buffer size."""
    nc = bacc.Bacc("TRN2", target_bir_lowering=False, debug=False,
                   num_devices=NCORES)
    src = nc.dram_tensor("src", [NCORES, HD, RPC], BF16,
                         kind="ExternalInput").ap()
    dst = nc.dram_tensor("dst", [1, 16], F32, kind="ExternalOutput").ap()
    with tile.TileContext(nc) as tc, ExitStack() as octx:
        dram = octx.enter_context(tc.tile_pool(name="dram", bufs=1,
                                               space="DRAM"))
        pool = octx.enter_context(tc.tile_pool(name="sb", bufs=1))
        a = dram.tile([NCORES, HD, RPC], BF16)
        bb = dram.tile([NCORES, HD, RPC], BF16)
        nc.sync.dma_start(a[:], src[:])
        bufs = [a, bb]
        for i in range(k):
            nc.gpsimd.collective_compute(
                "AllToAll", ALU.bypass,
                replica_groups=[list(range(NCORES))],
                ins=[bufs[i % 2][:]], outs=[bufs[(i + 1) % 2][:]],
            )
        dsb = pool.tile([1, 16], F32)
        nc.vector.memset(dsb[:], 0.0)
        nc.sync.dma_start(dst[:], dsb[:])
    nc.compile()
    return nc


def _make_runner(nc):
    """Jitted 8-core SPMD executor for a compiled Bass module."""
    install_neuronx_cc_hook()
    partition_name = nc.partition_id_tensor.name if nc.partition_id_tensor else None
    in_names, out_names, out_avals = [], [], []
    for alloc in nc.m.functions[0].allocations:
        if not isinstance(alloc, mybir.MemoryLocationSet):
            continue
        name = alloc.memorylocations[0].name
        if alloc.kind == "ExternalInput":
            if name != partition_name:
                in_names.append(name)
        elif alloc.kind == "ExternalOutput":
            out_names.append(name)
            out_avals.append(jax.core.ShapedArray(
                tuple(alloc.tensor_shape), mybir.dt.np(alloc.dtype)))
    n_params = len(in_names)
    n_outs = len(out_avals)
    all_in_names = list(in_names) + list(out_names)
    if partition_name is not None:
        all_in_names.append(partition_name)
    donate = tuple(range(n_params, n_params + n_outs))

    def _body(*args):
        operands = list(args)
        if partition_name is not None:
            operands.append(partition_id_tensor())
        return tuple(_bass_exec_p.bind(
            *operands,
            out_avals=tuple(out_avals),
            in_names=tuple(all_in_names),
            out_names=tuple(out_names),
            lowering_input_output_aliases=(),
            sim_require_finite=True,
            sim_require_nnan=True,
            nc=nc,
        ))

    devices = jax.devices()[:NCORES]
    mesh = Mesh(np.asarray(devices), ("core",))
    sharded = jax.jit(
        shard_map(_body, mesh=mesh,
                  in_specs=(PartitionSpec("core"),) * (n_params + n_outs),
                  out_specs=(PartitionSpec("core"),) * n_outs,
                  check_rep=False),
        donate_argnums=donate, keep_unused=True)

    zero_shapes = [a.shape for a in out_avals]
    zero_dtypes = [a.dtype for a in out_avals]

    def _zeros():
        return [np.zeros((NCORES * s[0], *s[1:]), d)
                for s, d in zip(zero_shapes, zero_dtypes)]

    def prepare(in_maps):
        """Concatenate per-core inputs and stage them on device once."""
        return [
            jax.device_put(np.concatenate(
                [np.asarray(m[name]) for m in in_maps], axis=0))
            for name in in_names
        ]

    def run_prepared(handles, as_numpy=True):
        out_arrs = sharded(*handles, *_zeros())
        if not as_numpy:
            jax.block_until_ready(out_arrs)
            return out_arrs
        return [
            {name: np.asarray(out_arrs[i]).reshape(NCORES, *zero_shapes[i])[c]
             for i, name in enumerate(out_names)}
            for c in range(NCORES)
        ]

    def run(in_maps):
        return run_prepared(prepare(in_maps))

    run.prepare = prepare
    run.run_prepared = run_prepared
    return run


def _shard_inputs(x, Wi, bi, Wo, bo, causal):
    """Host-side slicing/layout prep -> per-core input maps."""
    bf = ml_dtypes.bfloat16
    x = np.asarray(x, np.float32)
    Wi = np.asarray(Wi, np.float32)
    bi = np.asarray(bi, np.float32)
    Wo = np.asarray(Wo, np.float32)
    bo = np.asarray(bo, np.float32)

    xt = np.ascontiguousarray(x.reshape(R, D).T).astype(bf)       # (D, R)
    wot = np.ascontiguousarray(Wo.T).astype(bf)                   # (D, D)

    # additive triangle mask [128, 128]: NEG strictly below the diagonal
    # ([k, q] layout: masked iff q < k)
    m = np.zeros((KB, KB), np.float32)
    if causal:
        i = np.arange(KB)[:, None]
        j = np.arange(KB)[None, :]
        m[:, :] = np.where(j < i, NEG, 0.0)
    m = m.astype(bf)

    in_maps = []
    for c in range(NCORES):
        rows = np.concatenate([
            np.arange(c * HD, (c + 1) * HD),
            D + np.arange(c * HD, (c + 1) * HD),
            2 * D + np.arange(c * HD, (c + 1) * HD),
        ])
        wit_c = np.ascontiguousarray(Wi[rows].T).astype(bf)       # (D, 384)
        bi_c = np.ascontiguousarray(bi[rows]).astype(np.float32)  # (384,)
        in_maps.append({
            "xt": xt, "wit": wit_c, "bi_s": bi_c,
            "wot": wot, "bo_f": bo, "masks": m,
        })
    return in_maps


_CACHE = {}


def _get_runner(causal, repeat=1):
    key = (causal, repeat)
    if key not in _CACHE:
        nc = _build(causal, repeat)
        _CACHE[key] = _make_runner(nc)
    return _CACHE[key]


def kernel(x, Wi, bi, Wo, bo, causal_mask):
    causal = bool(int(np.asarray(causal_mask)))
    run = _get_runner(causal)
    in_maps = _shard_inputs(x, Wi, bi, Wo, bo, causal)
    res = run(in_maps)
    # res[c]["out_t"]: (D, RPC) fp32 = transposed rows [c*RPC, (c+1)*RPC)
    full = np.concatenate([res[c]["out_t"].T for c in range(NCORES)], axis=0)
    return np.ascontiguousarray(full.reshape(B, S, D).astype(np.float32))


# revision 10
# speedup vs baseline: 1.0073x; 1.0073x over previous
"""Multi-head causal attention (B=2, S=2048, D=1024, H=16) on 8 Trainium2 cores.

Sharding: tensor-parallel over heads. Core c computes QKV projection, causal
attention and softmax for heads {2c, 2c+1} over both batches, then an AllToAll
redistributes the attention output so core c owns rows [512c, 512c+512) of the
flattened (B*S, D) activation; each core applies the full output projection to
its row slice. Host code only slices/transposes inputs and concatenates the
per-core output slices.

All matmuls run in bf16 with fp32 PSUM accumulation. The pipeline works in
transposed layout ([dim, seq]) so that softmax reduces over the PSUM partition
axis via a ones-column folded into the PV matmul, and the attention output
lands directly in the layout the output projection consumes.

v2 notes:
- PSUM plan: tag "big" ([128,1024], 2 slots) + tags "o0"/"o1" ([128,512],
  2 slots each) = 8 banks. Everything (QKV chains, score pairs, PV
  accumulators, out-proj chains) is double-buffered, so no phase ever stalls
  the PE on a single-buffered drain (the v1 ps_o drain cost ~5us/query-block
  and re-throttled the PE clock each time).
- Causal masking is applied in PSUM via matmul: identity.T @ staircase adds
  -1e30 to masked score entries before the exp, so exp is always one full
  [128,1024] tile and there are no DVE mask-multiplies / gpsimd memsets.
- Softmax normalization uses reciprocal_approx_fast (~5x faster than the
  iterative-divide reciprocal which cost 3.3us per call on one partition).
- QKV bias+downcast runs on the (otherwise idle) scalar engine.
"""
import numpy as np
from contextlib import ExitStack

import jax
import ml_dtypes

import concourse.bass as bass
import concourse.tile as tile
from concourse import bacc, mybir
from concourse.bass2jax import (
    _bass_exec_p,
    install_neuronx_cc_hook,
    partition_id_tensor,
)
from jax.sharding import Mesh, PartitionSpec
from jax.experimental.shard_map import shard_map

B, S, D, H = 2, 2048, 1024, 16
DH = D // H            # 64
NCORES = 8
HPC = H // NCORES      # heads per core = 2
HD = HPC * DH          # head dims per core = 128
R = B * S              # flattened rows = 4096
RPC = R // NCORES      # rows per core after AllToAll = 512
QB = 512               # query block (also the AllToAll shard size)
KB = 128               # key block
NQB = S // QB          # 4 query blocks per batch
NKB = S // KB          # 16 key blocks per batch
CCH = D // 128         # contraction chunks for D-wide matmuls = 8

BF16 = mybir.dt.bfloat16
F32 = mybir.dt.float32
AF = mybir.ActivationFunctionType
ALU = mybir.AluOpType
NEG = -1e30            # added to masked score entries before exp


def _build(causal: bool, repeat: int = 1, loop_n: int = 0,
           a2a_local: bool = False, parts: str = "full"):
    """Emit the SPMD Bass program (identical on all 8 cores).

    loop_n > 0 builds a timing variant: the whole per-iteration body runs
    inside a hardware For_i loop and the AllToAll is replaced by a local DMA
    copy (collectives cannot sit inside control flow), with the real output
    replaced by a tiny dummy (so the timing loop's donated output buffers are
    negligible to transfer). Used only to measure per-iteration device time.
    """
    timing = loop_n > 0
    nc = bacc.Bacc("TRN2", target_bir_lowering=False, debug=False,
                   num_devices=NCORES)

    xt = nc.dram_tensor("xt", [D, R], BF16, kind="ExternalInput").ap()
    wit = nc.dram_tensor("wit", [D, 3 * HD], BF16, kind="ExternalInput").ap()
    bi_s = nc.dram_tensor("bi_s", [3 * HD], F32, kind="ExternalInput").ap()
    wot = nc.dram_tensor("wot", [D, D], BF16, kind="ExternalInput").ap()
    bo_f = nc.dram_tensor("bo_f", [D], F32, kind="ExternalInput").ap()
    masks = nc.dram_tensor("masks", [KB, KB], BF16, kind="ExternalInput").ap()
    if timing:
        out_t = nc.dram_tensor("out_scratch", [D, RPC], F32).ap()
        dummy = nc.dram_tensor("tiny_out", [1, 16], F32, kind="ExternalOutput").ap()
    else:
        out_t = nc.dram_tensor("out_t", [D, RPC], F32, kind="ExternalOutput").ap()

    with tile.TileContext(nc) as tc, ExitStack() as octx:
        persist = octx.enter_context(tc.tile_pool(name="persist", bufs=1))
        dram = octx.enter_context(tc.tile_pool(name="dram", bufs=1, space="DRAM"))

        # ---- persistent SBUF state (x chunks queued right after wit: the
        # QKV matmuls need them first; wot/bo only matter at the end) ----
        wit_sb = persist.tile([128, CCH, 3 * HD], BF16)
        nc.sync.dma_start(wit_sb[:], wit.rearrange("(cc p) n -> p cc n", p=128))
        bias_sb = persist.tile([128, 3], F32)
        nc.sync.dma_start(bias_sb[:], bi_s.rearrange("(t p) -> p t", p=128))
        xt_pool = octx.enter_context(tc.tile_pool(name="xt_pool", bufs=1))
        xt_sb = xt_pool.tile([128, CCH, R], BF16)
        xt_r = xt.rearrange("(cc p) r -> p cc r", p=128)
        for cc in range(CCH):
            nc.sync.dma_start(xt_sb[:, cc, :], xt_r[:, cc, :])
        wot_sb = persist.tile([128, CCH, D], BF16)
        nc.sync.dma_start(wot_sb[:], wot.rearrange("(cc p) o -> p cc o", p=128))
        bo_sb = persist.tile([128, CCH], F32)
        nc.sync.dma_start(bo_sb[:], bo_f.rearrange("(oc p) -> p oc", p=128))
        # additive triangle mask [128, 128]: NEG strictly below the diagonal
        # in [k, q] layout. identity.T @ tri adds NEG to the masked entries of
        # a diagonal 128x128 score sub-block before the exp.
        tri_sb = persist.tile([128, KB], BF16)
        if causal:
            nc.sync.dma_start(tri_sb[:], masks[:])

        identity = persist.tile([128, 128], BF16)
        from concourse.masks import make_identity
        make_identity(nc, identity[:])

        # qT/kT: [head-dims (2 heads x 64), S] per batch; v: [k rows, 65] blocks
        qt_sb = [persist.tile([128, S], BF16, name=f"qt{b}") for b in range(B)]
        kt_sb = [persist.tile([128, S], BF16, name=f"kt{b}") for b in range(B)]
        # v_sb[h][:, g, :]: col 0 = 1.0, cols 1..31 = 0, cols 32..95 = v rows
        # for global k-block g. The ones column puts the PV denominator on
        # PSUM partition 0 (aligned for the custom-DVE fast reciprocal); the
        # zero pad puts the output dims at partitions 32..95 (PSUM reads must
        # be 32-partition aligned).
        VP, VD0 = 128, 64
        v_sb = [persist.tile([128, B * NKB, VP], BF16, name=f"v{h}")
                for h in range(HPC)]
        for h in range(HPC):
            nc.vector.memset(v_sb[h][:, :, 0:VD0], 0.0)
            nc.vector.memset(v_sb[h][:, :, 0:1], 1.0)

        a2a_in = dram.tile([NCORES, HD, RPC], BF16)
        a2a_out = dram.tile([NCORES, HD, RPC], BF16)
        ao_sb = persist.tile([128, NCORES, RPC], BF16, name="ao_sb")

        # one PSUM pool; tags (all bufs=2, 8 banks total):
        #   big: [128,1024] x2 slots (4 banks)
        #   o0/o1: [128,512] x2 slots each (2+2 banks)
        psum = octx.enter_context(tc.tile_pool(name="psum", bufs=1,
                                               space="PSUM"))
        work = octx.enter_context(tc.tile_pool(name="work", bufs=3))
        epool = octx.enter_context(tc.tile_pool(name="epool", bufs=4))

        def big_ps(name):
            return psum.tile([128, 2 * QB], F32, tag="big", bufs=2, name=name)

        def o_ps(i, name, shape=None, dtype=F32):
            return psum.tile(shape or [128, QB], dtype, tag=f"o{i % 2}",
                             bufs=2, name=name)

        def emit_body(a2a_local: bool):
            # ================= QKV projection (transposed) =================
            for b in range(B):
                for tsr in range(3):  # 0=q, 1=k, 2=v
                    vt_full = None
                    if tsr == 2:
                        vt_full = work.tile([128, S], BF16, tag="vt",
                                            name=f"vt{b}")
                    # 4 concurrent 512-wide chains: big slot (2) + o0 + o1,
                    # double-buffered across passes so the previous pass's
                    # bias/downcast drain never stalls the PE.
                    ps_big = big_ps("ps_qkv")
                    ps_oo = [o_ps(i, "ps_qkvo") for i in range(2)]

                    def chain(rc):
                        if rc < 2:
                            return ps_big[:, rc * QB:(rc + 1) * QB]
                        return ps_oo[rc - 2][:]

                    for cc in range(CCH):
                        for rc in range(S // QB):
                            r0 = b * S + rc * QB
                            nc.tensor.matmul(
                                chain(rc),
                                wit_sb[:, cc, tsr * HD:(tsr + 1) * HD],
                                xt_sb[:, cc, r0:r0 + QB],
                                start=(cc == 0), stop=(cc == CCH - 1),
                            )
                    dst = (qt_sb[b] if tsr == 0 else
                           kt_sb[b] if tsr == 1 else vt_full)
                    # fused per-partition bias add + bf16 downcast, split
                    # across the scalar + vector engines so the pass-boundary
                    # drain latency is halved
                    nc.scalar.activation(dst[:, 0:2 * QB], ps_big[:],
                                         AF.Identity, bias=bias_sb[:, tsr:tsr + 1])
                    for i in range(2):
                        nc.vector.tensor_scalar(
                            dst[:, (2 + i) * QB:(3 + i) * QB], ps_oo[i][:],
                            bias_sb[:, tsr:tsr + 1], None, ALU.add)
                    if tsr == 2:
                        # transpose vT -> v blocks [k rows, dims] on the DMA
                        # crossbar (frees the PE transposes + DVE copies)
                        for h in range(HPC):
                            nc.sync.dma_start_transpose(
                                v_sb[h][:, b * NKB:(b + 1) * NKB,
                                        VD0:VD0 + DH],
                                vt_full[h * DH:(h + 1) * DH, :])

            # ======================= attention =============================
            if parts == "qkv":
                return
            # scores run one k-block ahead of PV so PE never waits on exp
            for b in range(B):
                for qb in range(NQB):
                    nkb = 4 * (qb + 1) if causal else NKB
                    q0 = qb * QB
                    ps_o = [o_ps(h, f"ps_o{h}", shape=[VP, QB])
                            for h in range(HPC)]

                    def diag_i(kb):
                        # index of kb within this query block's diagonal
                        # 512x512 region, or -1 if kb is fully unmasked
                        i = kb - 4 * qb
                        return i if (causal and 0 <= i < 4) else -1

                    def scores_pair(p):
                        """Two k-blocks (2p, 2p+1) -> one [128,1024] psum per
                        head. For diagonal k-blocks, identity.T @ tri adds NEG
                        to the 128x128 triangle in PSUM (so exp -> 0 there)
                        and the fully-masked query-column prefix is simply
                        skipped by the exp and the PV matmul (N-sliced). MMs
                        alternate heads so consecutive matmuls land on
                        different PE row-groups and overlap in the array."""
                        pss = [big_ps(f"ps_s{h}") for h in range(HPC)]
                        for half in range(2):
                            for h in range(HPC):
                                kb = 2 * p + half
                                nc.tensor.matmul(
                                    pss[h][:, half * QB:(half + 1) * QB],
                                    kt_sb[b][h * DH:(h + 1) * DH,
                                             kb * KB:(kb + 1) * KB],
                                    qt_sb[b][h * DH:(h + 1) * DH, q0:q0 + QB],
                                    start=True, stop=True,
                                )
                        if causal and "nodiag" not in parts:
                            for h in range(HPC):
                                for half in range(2):
                                    i = diag_i(2 * p + half)
                                    if i >= 0:
                                        c = half * QB + i * KB
                                        nc.tensor.matmul(
                                            pss[h][:, c:c + KB],
                                            identity[:], tri_sb[:],
                                            start=False, stop=True,
                                        )
                        # exp skips the fully-masked prefix of half 0 (the
                        # garbage it leaves in half 1's prefix is never read:
                        # the PV rhs is sliced identically)
                        i0 = diag_i(2 * p)
                        c0 = i0 * KB if (i0 > 0 and "nodiag" not in parts) \
                            else 0
                        es = []
                        for h in range(HPC):
                            e = epool.tile([128, 2 * QB], BF16, tag="expT",
                                           name="expT")
                            if "peonly" in parts:
                                es.append(None)
                                continue
                            nc.scalar.activation(e[:, c0:], pss[h][:, c0:],
                                                 AF.Exp, scale=1.0 / 8.0)
                            es.append(e)
                        return es

                    def pv_pair(p, es):
                        for h in range(HPC):
                            for half in range(2):
                                kb = 2 * p + half
                                i = diag_i(kb)
                                cs = i * KB if (i > 0 and
                                                "nodiag" not in parts) else 0
                                rhs = (qt_sb[b][:, q0 + cs:q0 + QB]
                                       if es[h] is None else
                                       es[h][:, half * QB + cs:
                                             (half + 1) * QB])
                                nc.tensor.matmul(
                                    ps_o[h][:, cs:],
                                    v_sb[h][:, b * NKB + kb, :],
                                    rhs,
                                    start=(kb == 0), stop=(kb == nkb - 1),
                                )

                    npair = nkb // 2
                    es_prev = scores_pair(0)
                    for p in range(1, npair):
                        es = scores_pair(p)
                        pv_pair(p - 1, es_prev)
                        es_prev = es
                    pv_pair(npair - 1, es_prev)

                    at = work.tile([128, QB], BF16, tag="attnT", name="attnT")
                    if "nonorm" in parts:
                        nc.vector.tensor_copy(at[0:DH, :],
                                              ps_o[0][VD0:VD0 + DH, :])
                        nc.vector.tensor_copy(at[DH:2 * DH, :],
                                              ps_o[1][VD0:VD0 + DH, :])
                    else:
                        # normalize: fast approximate reciprocal of the
                        # denominators (ones-column row 0 of each PV psum,
                        # partition-aligned for the custom-DVE op),
                        # replicated to all partitions on GPSIMD
                        rc = work.tile([1, 2 * QB], F32, tag="rc", name="rc")
                        for h in range(HPC):
                            nc.vector.reciprocal_approx_fast(
                                rc[0:1, h * QB:(h + 1) * QB],
                                ps_o[h][0:1, :])
                        rpb = work.tile([128, 2 * QB], F32, tag="rpb",
                                        name="rpb")
                        nc.gpsimd.partition_broadcast(rpb[:], rc[0:1, :])
                        for h in range(HPC):
                            nc.vector.tensor_mul(
                                at[h * DH:(h + 1) * DH, :],
                                ps_o[h][VD0:VD0 + DH, :],
                                rpb[h * DH:(h + 1) * DH,
                                    h * QB:(h + 1) * QB])
                    j = b * NQB + qb
                    nc.sync.dma_start(a2a_in[j], at[:])
                    if a2a_local:
                        # timing stand-in for the exchange, overlapped with
                        # the rest of attention (collectives can't sit in
                        # control flow)
                        nc.sync.dma_start(a2a_out[j], a2a_in[j])
                        nc.sync.dma_start(ao_sb[:, j, :], a2a_out[j])

            # ================= AllToAll + output projection ================
            if parts == "qkv+att":
                return
            if not a2a_local:
                nc.gpsimd.collective_compute(
                    "AllToAll", ALU.bypass,
                    replica_groups=[list(range(NCORES))],
                    ins=[a2a_in[:]], outs=[a2a_out[:]],
                )
                for j in range(NCORES):
                    nc.sync.dma_start(ao_sb[:, j, :], a2a_out[j])
            # all 8 output chains live at once across the 8 PSUM banks,
            # contraction chunk j outermost: 56 of 64 matmuls can run before
            # the last exchanged shard lands
            pair_ps = [big_ps(f"ps_outp{i}") for i in range(2)]
            single_ps = [o_ps(i, f"ps_outs{i}", shape=[128, RPC])
                         for i in range(4)]

            def ob_slice(ob):
                if ob < 4:
                    return pair_ps[ob // 2][:, (ob % 2) * RPC:
                                            (ob % 2 + 1) * RPC]
                return single_ps[ob - 4][:]

            for j in range(NCORES):
                for ob in range(CCH):
                    nc.tensor.matmul(
                        ob_slice(ob),
                        wot_sb[:, j, ob * 128:(ob + 1) * 128],
                        ao_sb[:, j, :],
                        start=(j == 0), stop=(j == NCORES - 1),
                    )
            for ob in range(CCH):
                os = work.tile([128, RPC], F32, tag="os", name="os")
                # split the bias-add drain across scalar + vector engines
                if ob % 2:
                    nc.scalar.activation(os[:], ob_slice(ob), AF.Identity,
                                         bias=bo_sb[:, ob:ob + 1])
                else:
                    nc.vector.tensor_scalar(os[:], ob_slice(ob),
                                            bo_sb[:, ob:ob + 1], None, ALU.add)
                nc.sync.dma_start(out_t[ob * 128:(ob + 1) * 128, :], os[:])

        if loop_n:
            with tc.For_i(0, loop_n, 1,
                          hint_engines=(mybir.EngineType.PE,
                                        mybir.EngineType.DVE,
                                        mybir.EngineType.Activation)):
                emit_body(a2a_local=True)
            dsb = persist.tile([1, 16], F32)
            nc.vector.memset(dsb[:], 0.0)
            nc.sync.dma_start(dummy[:], dsb[:])
        else:
            for _ in range(repeat):
                emit_body(a2a_local=a2a_local)

    nc.compile()
    return nc


def _build_a2a_bench(k: int):
    """k back-to-back AllToAlls on the kernel's exchange buffer size."""
    nc = bacc.Bacc("TRN2", target_bir_lowering=False, debug=False,
                   num_devices=NCORES)
    src = nc.dram_tensor("src", [NCORES, HD, RPC], BF16,
                         kind="ExternalInput").ap()
    dst = nc.dram_tensor("dst", [1, 16], F32, kind="ExternalOutput").ap()
    with tile.TileContext(nc) as tc, ExitStack() as octx:
        dram = octx.enter_context(tc.tile_pool(name="dram", bufs=1,
                                               space="DRAM"))
        pool = octx.enter_context(tc.tile_pool(name="sb", bufs=1))
        a = dram.tile([NCORES, HD, RPC], BF16)
        bb = dram.tile([NCORES, HD, RPC], BF16)
        nc.sync.dma_start(a[:], src[:])
        bufs = [a, bb]
        for i in range(k):
            nc.gpsimd.collective_compute(
                "AllToAll", ALU.bypass,
                replica_groups=[list(range(NCORES))],
                ins=[bufs[i % 2][:]], outs=[bufs[(i + 1) % 2][:]],
            )
        dsb = pool.tile([1, 16], F32)
        nc.vector.memset(dsb[:], 0.0)
        nc.sync.dma_start(dst[:], dsb[:])
    nc.compile()
    return nc


def _make_runner(nc):
    """Jitted 8-core SPMD executor for a compiled Bass module."""
    install_neuronx_cc_hook()
    partition_name = nc.partition_id_tensor.name if nc.partition_id_tensor else None
    in_names, out_names, out_avals = [], [], []
    for alloc in nc.m.functions[0].allocations:
        if not isinstance(alloc, mybir.MemoryLocationSet):
            continue
        name = alloc.memorylocations[0].name
        if alloc.kind == "ExternalInput":
            if name != partition_name:
                in_names.append(name)
        elif alloc.kind == "ExternalOutput":
            out_names.append(name)
            out_avals.append(jax.core.ShapedArray(
                tuple(alloc.tensor_shape), mybir.dt.np(alloc.dtype)))
    n_params = len(in_names)
    n_outs = len(out_avals)
    all_in_names = list(in_names) + list(out_names)
    if partition_name is not None:
        all_in_names.append(partition_name)
    donate = tuple(range(n_params, n_params + n_outs))

    def _body(*args):
        operands = list(args)
        if partition_name is not None:
            operands.append(partition_id_tensor())
        return tuple(_bass_exec_p.bind(
            *operands,
            out_avals=tuple(out_avals),
            in_names=tuple(all_in_names),
            out_names=tuple(out_names),
            lowering_input_output_aliases=(),
            sim_require_finite=True,
            sim_require_nnan=True,
            nc=nc,
        ))

    devices = jax.devices()[:NCORES]
    mesh = Mesh(np.asarray(devices), ("core",))
    sharded = jax.jit(
        shard_map(_body, mesh=mesh,
                  in_specs=(PartitionSpec("core"),) * (n_params + n_outs),
                  out_specs=(PartitionSpec("core"),) * n_outs,
                  check_rep=False),
        donate_argnums=donate, keep_unused=True)

    zero_shapes = [a.shape for a in out_avals]
    zero_dtypes = [a.dtype for a in out_avals]

    def _zeros():
        return [np.zeros((NCORES * s[0], *s[1:]), d)
                for s, d in zip(zero_shapes, zero_dtypes)]

    def prepare(in_maps):
        """Concatenate per-core inputs and stage them on device once."""
        return [
            jax.device_put(np.concatenate(
                [np.asarray(m[name]) for m in in_maps], axis=0))
            for name in in_names
        ]

    def run_prepared(handles, as_numpy=True):
        out_arrs = sharded(*handles, *_zeros())
        if not as_numpy:
            jax.block_until_ready(out_arrs)
            return out_arrs
        return [
            {name: np.asarray(out_arrs[i]).reshape(NCORES, *zero_shapes[i])[c]
             for i, name in enumerate(out_names)}
            for c in range(NCORES)
        ]

    def run(in_maps):
        return run_prepared(prepare(in_maps))

    run.prepare = prepare
    run.run_prepared = run_prepared
    return run


def _shard_inputs(x, Wi, bi, Wo, bo, causal):
    """Host-side slicing/layout prep -> per-core input maps."""
    bf = ml_dtypes.bfloat16
    x = np.asarray(x, np.float32)
    Wi = np.asarray(Wi, np.float32)
    bi = np.asarray(bi, np.float32)
    Wo = np.asarray(Wo, np.float32)
    bo = np.asarray(bo, np.float32)

    xt = np.ascontiguousarray(x.reshape(R, D).T).astype(bf)       # (D, R)
    wot = np.ascontiguousarray(Wo.T).astype(bf)                   # (D, D)

    # additive triangle mask [128, 128]: NEG strictly below the diagonal
    # ([k, q] layout: masked iff q < k)
    m = np.zeros((KB, KB), np.float32)
    if causal:
        i = np.arange(KB)[:, None]
        j = np.arange(KB)[None, :]
        m[:, :] = np.where(j < i, NEG, 0.0)
    m = m.astype(bf)

    in_maps = []
    for c in range(NCORES):
        rows = np.concatenate([
            np.arange(c * HD, (c + 1) * HD),
            D + np.arange(c * HD, (c + 1) * HD),
            2 * D + np.arange(c * HD, (c + 1) * HD),
        ])
        wit_c = np.ascontiguousarray(Wi[rows].T).astype(bf)       # (D, 384)
        bi_c = np.ascontiguousarray(bi[rows]).astype(np.float32)  # (384,)
        in_maps.append({
            "xt": xt, "wit": wit_c, "bi_s": bi_c,
            "wot": wot, "bo_f": bo, "masks": m,
        })
    return in_maps


_CACHE = {}


def _get_runner(causal, repeat=1):
    key = (causal, repeat)
    if key not in _CACHE:
        nc = _build(causal, repeat)
        _CACHE[key] = _make_runner(nc)
    return _CACHE[key]


def kernel(x, Wi, bi, Wo, bo, causal_mask):
    causal = bool(int(np.asarray(causal_mask)))
    run = _get_runner(causal)
    in_maps = _shard_inputs(x, Wi, bi, Wo, bo, causal)
    res = run(in_maps)
    # res[c]["out_t"]: (D, RPC) fp32 = transposed rows [c*RPC, (c+1)*RPC)
    full = np.concatenate([res[c]["out_t"].T for c in range(NCORES)], axis=0)
    return np.ascontiguousarray(full.reshape(B, S, D).astype(np.float32))


# revision 11
# speedup vs baseline: 1.0294x; 1.0220x over previous
"""Multi-head causal attention (B=2, S=2048, D=1024, H=16) on 8 Trainium2 cores.

Sharding: tensor-parallel over heads. Core c computes QKV projection, causal
attention and softmax for heads {2c, 2c+1} over both batches, then an AllToAll
redistributes the attention output so core c owns rows [512c, 512c+512) of the
flattened (B*S, D) activation; each core applies the full output projection to
its row slice. Host code only slices/transposes inputs and concatenates the
per-core output slices.

All matmuls run in bf16 with fp32 PSUM accumulation. The pipeline works in
transposed layout ([dim, seq]) so that softmax reduces over the PSUM partition
axis via a ones-column folded into the PV matmul, and the attention output
lands directly in the layout the output projection consumes.

v2 notes:
- PSUM plan: tag "big" ([128,1024], 2 slots) + tags "o0"/"o1" ([128,512],
  2 slots each) = 8 banks. Everything (QKV chains, score pairs, PV
  accumulators, out-proj chains) is double-buffered, so no phase ever stalls
  the PE on a single-buffered drain (the v1 ps_o drain cost ~5us/query-block
  and re-throttled the PE clock each time).
- Causal masking is applied in PSUM via matmul: identity.T @ staircase adds
  -1e30 to masked score entries before the exp, so exp is always one full
  [128,1024] tile and there are no DVE mask-multiplies / gpsimd memsets.
- Softmax normalization uses reciprocal_approx_fast (~5x faster than the
  iterative-divide reciprocal which cost 3.3us per call on one partition).
- QKV bias+downcast runs on the (otherwise idle) scalar engine.
"""
import numpy as np
from contextlib import ExitStack

import jax
import ml_dtypes

import concourse.bass as bass
import concourse.tile as tile
from concourse import bacc, mybir
from concourse.bass2jax import (
    _bass_exec_p,
    install_neuronx_cc_hook,
    partition_id_tensor,
)
from jax.sharding import Mesh, PartitionSpec
from jax.experimental.shard_map import shard_map

B, S, D, H = 2, 2048, 1024, 16
DH = D // H            # 64
NCORES = 8
HPC = H // NCORES      # heads per core = 2
HD = HPC * DH          # head dims per core = 128
R = B * S              # flattened rows = 4096
RPC = R // NCORES      # rows per core after AllToAll = 512
QB = 512               # query block (also the AllToAll shard size)
KB = 128               # key block
NQB = S // QB          # 4 query blocks per batch
NKB = S // KB          # 16 key blocks per batch
CCH = D // 128         # contraction chunks for D-wide matmuls = 8

BF16 = mybir.dt.bfloat16
F32 = mybir.dt.float32
AF = mybir.ActivationFunctionType
ALU = mybir.AluOpType
NEG = -1e30            # added to masked score entries before exp


def _build(causal: bool, repeat: int = 1, loop_n: int = 0,
           a2a_local: bool = False, parts: str = "full"):
    """Emit the SPMD Bass program (identical on all 8 cores).

    loop_n > 0 builds a timing variant: the whole per-iteration body runs
    inside a hardware For_i loop and the AllToAll is replaced by a local DMA
    copy (collectives cannot sit inside control flow), with the real output
    replaced by a tiny dummy (so the timing loop's donated output buffers are
    negligible to transfer). Used only to measure per-iteration device time.
    """
    timing = loop_n > 0
    nc = bacc.Bacc("TRN2", target_bir_lowering=False, debug=False,
                   num_devices=NCORES)

    xt = nc.dram_tensor("xt", [D, R], BF16, kind="ExternalInput").ap()
    wit = nc.dram_tensor("wit", [D, 3 * HD], BF16, kind="ExternalInput").ap()
    bi_s = nc.dram_tensor("bi_s", [3 * HD], F32, kind="ExternalInput").ap()
    wot = nc.dram_tensor("wot", [D, D], BF16, kind="ExternalInput").ap()
    bo_f = nc.dram_tensor("bo_f", [D], F32, kind="ExternalInput").ap()
    masks = nc.dram_tensor("masks", [KB, KB], BF16, kind="ExternalInput").ap()
    if timing:
        out_t = nc.dram_tensor("out_scratch", [D, RPC], F32).ap()
        dummy = nc.dram_tensor("tiny_out", [1, 16], F32, kind="ExternalOutput").ap()
    else:
        out_t = nc.dram_tensor("out_t", [D, RPC], F32, kind="ExternalOutput").ap()

    with tile.TileContext(nc) as tc, ExitStack() as octx:
        persist = octx.enter_context(tc.tile_pool(name="persist", bufs=1))
        dram = octx.enter_context(tc.tile_pool(name="dram", bufs=1, space="DRAM"))

        # ---- persistent SBUF state (x chunks queued right after wit: the
        # QKV matmuls need them first; wot/bo only matter at the end) ----
        wit_sb = persist.tile([128, CCH, 3 * HD], BF16)
        nc.sync.dma_start(wit_sb[:], wit.rearrange("(cc p) n -> p cc n", p=128))
        bias_sb = persist.tile([128, 3], F32)
        nc.sync.dma_start(bias_sb[:], bi_s.rearrange("(t p) -> p t", p=128))
        xt_pool = octx.enter_context(tc.tile_pool(name="xt_pool", bufs=1))
        xt_sb = xt_pool.tile([128, CCH, R], BF16)
        xt_r = xt.rearrange("(cc p) r -> p cc r", p=128)
        for cc in range(CCH):
            nc.sync.dma_start(xt_sb[:, cc, :], xt_r[:, cc, :])
        wot_sb = persist.tile([128, CCH, D], BF16)
        nc.sync.dma_start(wot_sb[:], wot.rearrange("(cc p) o -> p cc o", p=128))
        bo_sb = persist.tile([128, CCH], F32)
        nc.sync.dma_start(bo_sb[:], bo_f.rearrange("(oc p) -> p oc", p=128))
        # additive triangle mask [128, 128]: NEG strictly below the diagonal
        # in [k, q] layout. identity.T @ tri adds NEG to the masked entries of
        # a diagonal 128x128 score sub-block before the exp.
        tri_sb = persist.tile([128, KB], BF16)
        if causal:
            nc.sync.dma_start(tri_sb[:], masks[:])

        identity = persist.tile([128, 128], BF16)
        from concourse.masks import make_identity
        make_identity(nc, identity[:])

        # qT/kT: [head-dims (2 heads x 64), S] per batch; v: [k rows, 65] blocks
        qt_sb = [persist.tile([128, S], BF16, name=f"qt{b}") for b in range(B)]
        kt_sb = [persist.tile([128, S], BF16, name=f"kt{b}") for b in range(B)]
        # v_sb[h][:, g, :]: col 0 = 1.0, cols 1..31 = 0, cols 32..95 = v rows
        # for global k-block g. The ones column puts the PV denominator on
        # PSUM partition 0 (aligned for the custom-DVE fast reciprocal); the
        # zero pad puts the output dims at partitions 32..95 (PSUM reads must
        # be 32-partition aligned).
        VP, VD0 = 128, 64
        v_sb = [persist.tile([128, B * NKB, VP], BF16, name=f"v{h}")
                for h in range(HPC)]
        for h in range(HPC):
            nc.vector.memset(v_sb[h][:, :, 0:VD0], 0.0)
            nc.vector.memset(v_sb[h][:, :, 0:1], 1.0)

        a2a_in = dram.tile([NCORES, HD, RPC], BF16)
        a2a_out = dram.tile([NCORES, HD, RPC], BF16)
        ao_sb = persist.tile([128, NCORES, RPC], BF16, name="ao_sb")

        # one PSUM pool; tags (all bufs=2, 8 banks total):
        #   big: [128,1024] x2 slots (4 banks)
        #   o0/o1: [128,512] x2 slots each (2+2 banks)
        psum = octx.enter_context(tc.tile_pool(name="psum", bufs=1,
                                               space="PSUM"))
        work = octx.enter_context(tc.tile_pool(name="work", bufs=3))
        epool = octx.enter_context(tc.tile_pool(name="epool", bufs=4))

        def big_ps(name):
            return psum.tile([128, 2 * QB], F32, tag="big", bufs=2, name=name)

        def o_ps(i, name, shape=None, dtype=F32):
            return psum.tile(shape or [128, QB], dtype, tag=f"o{i % 2}",
                             bufs=2, name=name)

        def emit_body(a2a_local: bool):
            # ================= QKV projection (transposed) =================
            for b in range(B):
                for tsr in range(3):  # 0=q, 1=k, 2=v
                    vt_full = None
                    if tsr == 2:
                        vt_full = work.tile([128, S], BF16, tag="vt",
                                            name=f"vt{b}")
                    # 4 concurrent 512-wide chains: big slot (2) + o0 + o1,
                    # double-buffered across passes so the previous pass's
                    # bias/downcast drain never stalls the PE.
                    ps_big = big_ps("ps_qkv")
                    ps_oo = [o_ps(i, "ps_qkvo") for i in range(2)]

                    def chain(rc):
                        if rc < 2:
                            return ps_big[:, rc * QB:(rc + 1) * QB]
                        return ps_oo[rc - 2][:]

                    for cc in range(CCH):
                        for rc in range(S // QB):
                            r0 = b * S + rc * QB
                            nc.tensor.matmul(
                                chain(rc),
                                wit_sb[:, cc, tsr * HD:(tsr + 1) * HD],
                                xt_sb[:, cc, r0:r0 + QB],
                                start=(cc == 0), stop=(cc == CCH - 1),
                            )
                    dst = (qt_sb[b] if tsr == 0 else
                           kt_sb[b] if tsr == 1 else vt_full)
                    # fused per-partition bias add + bf16 downcast, split
                    # across the scalar + vector engines so the pass-boundary
                    # drain latency is halved
                    nc.scalar.activation(dst[:, 0:2 * QB], ps_big[:],
                                         AF.Identity, bias=bias_sb[:, tsr:tsr + 1])
                    for i in range(2):
                        nc.vector.tensor_scalar(
                            dst[:, (2 + i) * QB:(3 + i) * QB], ps_oo[i][:],
                            bias_sb[:, tsr:tsr + 1], None, ALU.add)
                    if tsr == 2:
                        # transpose vT -> v blocks [k rows, dims] on the DMA
                        # crossbar (frees the PE transposes + DVE copies)
                        for h in range(HPC):
                            nc.sync.dma_start_transpose(
                                v_sb[h][:, b * NKB:(b + 1) * NKB,
                                        VD0:VD0 + DH],
                                vt_full[h * DH:(h + 1) * DH, :])

            # ======================= attention =============================
            if parts == "qkv":
                return
            # scores run one k-block ahead of PV so PE never waits on exp
            for b in range(B):
                for qb in range(NQB):
                    nkb = 4 * (qb + 1) if causal else NKB
                    q0 = qb * QB
                    ps_o = [o_ps(h, f"ps_o{h}", shape=[VP, QB])
                            for h in range(HPC)]

                    def diag_i(kb):
                        # index of kb within this query block's diagonal
                        # 512x512 region, or -1 if kb is fully unmasked
                        i = kb - 4 * qb
                        return i if (causal and 0 <= i < 4) else -1

                    def scores_pair(p):
                        """Two k-blocks (2p, 2p+1) -> one [128,1024] psum per
                        head. For diagonal k-blocks, identity.T @ tri adds NEG
                        to the 128x128 triangle in PSUM (so exp -> 0 there)
                        and the fully-masked query-column prefix is simply
                        skipped by the exp and the PV matmul (N-sliced). MMs
                        alternate heads so consecutive matmuls land on
                        different PE row-groups and overlap in the array."""
                        pss = [big_ps(f"ps_s{h}") for h in range(HPC)]
                        for half in range(2):
                            for h in range(HPC):
                                kb = 2 * p + half
                                nc.tensor.matmul(
                                    pss[h][:, half * QB:(half + 1) * QB],
                                    kt_sb[b][h * DH:(h + 1) * DH,
                                             kb * KB:(kb + 1) * KB],
                                    qt_sb[b][h * DH:(h + 1) * DH, q0:q0 + QB],
                                    start=True, stop=True,
                                )
                        if causal and "nodiag" not in parts:
                            for h in range(HPC):
                                for half in range(2):
                                    i = diag_i(2 * p + half)
                                    if i >= 0:
                                        c = half * QB + i * KB
                                        nc.tensor.matmul(
                                            pss[h][:, c:c + KB],
                                            identity[:], tri_sb[:],
                                            start=False, stop=True,
                                        )
                        # exp skips the fully-masked prefix of half 0 (the
                        # garbage it leaves in half 1's prefix is never read:
                        # the PV rhs is sliced identically)
                        i0 = diag_i(2 * p)
                        c0 = i0 * KB if (i0 > 0 and "nodiag" not in parts) \
                            else 0
                        es = []
                        for h in range(HPC):
                            e = epool.tile([128, 2 * QB], BF16, tag="expT",
                                           name="expT")
                            if "peonly" in parts:
                                es.append(None)
                                continue
                            nc.scalar.activation(e[:, c0:], pss[h][:, c0:],
                                                 AF.Exp, scale=1.0 / 8.0)
                            es.append(e)
                        return es

                    def pv_pair(p, es):
                        for h in range(HPC):
                            for half in range(2):
                                kb = 2 * p + half
                                i = diag_i(kb)
                                cs = i * KB if (i > 0 and
                                                "nodiag" not in parts) else 0
                                rhs = (qt_sb[b][:, q0 + cs:q0 + QB]
                                       if es[h] is None else
                                       es[h][:, half * QB + cs:
                                             (half + 1) * QB])
                                nc.tensor.matmul(
                                    ps_o[h][:, cs:],
                                    v_sb[h][:, b * NKB + kb, :],
                                    rhs,
                                    start=(kb == 0), stop=(kb == nkb - 1),
                                )

                    npair = nkb // 2
                    es_prev = scores_pair(0)
                    for p in range(1, npair):
                        es = scores_pair(p)
                        pv_pair(p - 1, es_prev)
                        es_prev = es
                    pv_pair(npair - 1, es_prev)

                    at = work.tile([128, QB], BF16, tag="attnT", name="attnT")
                    if "nonorm" in parts:
                        nc.vector.tensor_copy(at[0:DH, :],
                                              ps_o[0][VD0:VD0 + DH, :])
                        nc.vector.tensor_copy(at[DH:2 * DH, :],
                                              ps_o[1][VD0:VD0 + DH, :])
                    else:
                        # normalize: fast approximate reciprocal of the
                        # denominators (ones-column row 0 of each PV psum,
                        # partition-aligned for the custom-DVE op),
                        # replicated to all partitions on GPSIMD
                        rc = work.tile([1, 2 * QB], F32, tag="rc", name="rc")
                        for h in range(HPC):
                            nc.vector.reciprocal_approx_fast(
                                rc[0:1, h * QB:(h + 1) * QB],
                                ps_o[h][0:1, :])
                        rpb = work.tile([128, 2 * QB], F32, tag="rpb",
                                        name="rpb")
                        nc.gpsimd.partition_broadcast(rpb[:], rc[0:1, :])
                        for h in range(HPC):
                            nc.vector.tensor_mul(
                                at[h * DH:(h + 1) * DH, :],
                                ps_o[h][VD0:VD0 + DH, :],
                                rpb[h * DH:(h + 1) * DH,
                                    h * QB:(h + 1) * QB])
                    j = b * NQB + qb
                    nc.sync.dma_start(a2a_in[j], at[:])
                    if a2a_local:
                        # timing stand-in for the exchange, overlapped with
                        # the rest of attention (collectives can't sit in
                        # control flow)
                        nc.sync.dma_start(a2a_out[j], a2a_in[j])
                        nc.sync.dma_start(ao_sb[:, j, :], a2a_out[j])

            # ================= AllToAll + output projection ================
            if parts == "qkv+att":
                return
            if not a2a_local:
                nc.gpsimd.collective_compute(
                    "AllToAll", ALU.bypass,
                    replica_groups=[list(range(NCORES))],
                    ins=[a2a_in[:]], outs=[a2a_out[:]],
                )
                for j in range(NCORES):
                    nc.sync.dma_start(ao_sb[:, j, :], a2a_out[j])
            # all 8 output chains live at once across the 8 PSUM banks,
            # contraction chunk j outermost: 56 of 64 matmuls can run before
            # the last exchanged shard lands
            pair_ps = [big_ps(f"ps_outp{i}") for i in range(2)]
            single_ps = [o_ps(i, f"ps_outs{i}", shape=[128, RPC])
                         for i in range(4)]

            def ob_slice(ob):
                if ob < 4:
                    return pair_ps[ob // 2][:, (ob % 2) * RPC:
                                            (ob % 2 + 1) * RPC]
                return single_ps[ob - 4][:]

            for j in range(NCORES):
                for ob in range(CCH):
                    nc.tensor.matmul(
                        ob_slice(ob),
                        wot_sb[:, j, ob * 128:(ob + 1) * 128],
                        ao_sb[:, j, :],
                        start=(j == 0), stop=(j == NCORES - 1),
                    )
            # drain order frees the PSUM slots the next iteration's first QKV
            # pass reuses (big slot 0 = ob 0/1, o0/o1 slot a = ob 4/5) first;
            # bias-add split across scalar + vector engines
            for k, ob in enumerate((0, 1, 4, 5, 2, 3, 6, 7)):
                os = work.tile([128, RPC], F32, tag="os", name="os")
                if k % 2:
                    nc.scalar.activation(os[:], ob_slice(ob), AF.Identity,
                                         bias=bo_sb[:, ob:ob + 1])
                else:
                    nc.vector.tensor_scalar(os[:], ob_slice(ob),
                                            bo_sb[:, ob:ob + 1], None, ALU.add)
                nc.sync.dma_start(out_t[ob * 128:(ob + 1) * 128, :], os[:])

        if loop_n:
            with tc.For_i(0, loop_n, 1,
                          hint_engines=(mybir.EngineType.PE,
                                        mybir.EngineType.DVE,
                                        mybir.EngineType.Activation)):
                emit_body(a2a_local=True)
            dsb = persist.tile([1, 16], F32)
            nc.vector.memset(dsb[:], 0.0)
            nc.sync.dma_start(dummy[:], dsb[:])
        else:
            for _ in range(repeat):
                emit_body(a2a_local=a2a_local)

    nc.compile()
    return nc


def _build_a2a_bench(k: int):
    """k back-to-back AllToAlls on the kernel's exchange buffer size."""
    nc = bacc.Bacc("TRN2", target_bir_lowering=False, debug=False,
                   num_devices=NCORES)
    src = nc.dram_tensor("src", [NCORES, HD, RPC], BF16,
                         kind="ExternalInput").ap()
    dst = nc.dram_tensor("dst", [1, 16], F32, kind="ExternalOutput").ap()
    with tile.TileContext(nc) as tc, ExitStack() as octx:
        dram = octx.enter_context(tc.tile_pool(name="dram", bufs=1,
                                               space="DRAM"))
        pool = octx.enter_context(tc.tile_pool(name="sb", bufs=1))
        a = dram.tile([NCORES, HD, RPC], BF16)
        bb = dram.tile([NCORES, HD, RPC], BF16)
        nc.sync.dma_start(a[:], src[:])
        bufs = [a, bb]
        for i in range(k):
            nc.gpsimd.collective_compute(
                "AllToAll", ALU.bypass,
                replica_groups=[list(range(NCORES))],
                ins=[bufs[i % 2][:]], outs=[bufs[(i + 1) % 2][:]],
            )
        dsb = pool.tile([1, 16], F32)
        nc.vector.memset(dsb[:], 0.0)
        nc.sync.dma_start(dst[:], dsb[:])
    nc.compile()
    return nc


def _make_runner(nc):
    """Jitted 8-core SPMD executor for a compiled Bass module."""
    install_neuronx_cc_hook()
    partition_name = nc.partition_id_tensor.name if nc.partition_id_tensor else None
    in_names, out_names, out_avals = [], [], []
    for alloc in nc.m.functions[0].allocations:
        if not isinstance(alloc, mybir.MemoryLocationSet):
            continue
        name = alloc.memorylocations[0].name
        if alloc.kind == "ExternalInput":
            if name != partition_name:
                in_names.append(name)
        elif alloc.kind == "ExternalOutput":
            out_names.append(name)
            out_avals.append(jax.core.ShapedArray(
                tuple(alloc.tensor_shape), mybir.dt.np(alloc.dtype)))
    n_params = len(in_names)
    n_outs = len(out_avals)
    all_in_names = list(in_names) + list(out_names)
    if partition_name is not None:
        all_in_names.append(partition_name)
    donate = tuple(range(n_params, n_params + n_outs))

    def _body(*args):
        operands = list(args)
        if partition_name is not None:
            operands.append(partition_id_tensor())
        return tuple(_bass_exec_p.bind(
            *operands,
            out_avals=tuple(out_avals),
            in_names=tuple(all_in_names),
            out_names=tuple(out_names),
            lowering_input_output_aliases=(),
            sim_require_finite=True,
            sim_require_nnan=True,
            nc=nc,
        ))

    devices = jax.devices()[:NCORES]
    mesh = Mesh(np.asarray(devices), ("core",))
    sharded = jax.jit(
        shard_map(_body, mesh=mesh,
                  in_specs=(PartitionSpec("core"),) * (n_params + n_outs),
                  out_specs=(PartitionSpec("core"),) * n_outs,
                  check_rep=False),
        donate_argnums=donate, keep_unused=True)

    zero_shapes = [a.shape for a in out_avals]
    zero_dtypes = [a.dtype for a in out_avals]

    def _zeros():
        return [np.zeros((NCORES * s[0], *s[1:]), d)
                for s, d in zip(zero_shapes, zero_dtypes)]

    def prepare(in_maps):
        """Concatenate per-core inputs and stage them on device once."""
        return [
            jax.device_put(np.concatenate(
                [np.asarray(m[name]) for m in in_maps], axis=0))
            for name in in_names
        ]

    def run_prepared(handles, as_numpy=True):
        out_arrs = sharded(*handles, *_zeros())
        if not as_numpy:
            jax.block_until_ready(out_arrs)
            return out_arrs
        return [
            {name: np.asarray(out_arrs[i]).reshape(NCORES, *zero_shapes[i])[c]
             for i, name in enumerate(out_names)}
            for c in range(NCORES)
        ]

    def run(in_maps):
        return run_prepared(prepare(in_maps))

    run.prepare = prepare
    run.run_prepared = run_prepared
    return run


def _shard_inputs(x, Wi, bi, Wo, bo, causal):
    """Host-side slicing/layout prep -> per-core input maps."""
    bf = ml_dtypes.bfloat16
    x = np.asarray(x, np.float32)
    Wi = np.asarray(Wi, np.float32)
    bi = np.asarray(bi, np.float32)
    Wo = np.asarray(Wo, np.float32)
    bo = np.asarray(bo, np.float32)

    xt = np.ascontiguousarray(x.reshape(R, D).T).astype(bf)       # (D, R)
    wot = np.ascontiguousarray(Wo.T).astype(bf)                   # (D, D)

    # additive triangle mask [128, 128]: NEG strictly below the diagonal
    # ([k, q] layout: masked iff q < k)
    m = np.zeros((KB, KB), np.float32)
    if causal:
        i = np.arange(KB)[:, None]
        j = np.arange(KB)[None, :]
        m[:, :] = np.where(j < i, NEG, 0.0)
    m = m.astype(bf)

    in_maps = []
    for c in range(NCORES):
        rows = np.concatenate([
            np.arange(c * HD, (c + 1) * HD),
            D + np.arange(c * HD, (c + 1) * HD),
            2 * D + np.arange(c * HD, (c + 1) * HD),
        ])
        wit_c = np.ascontiguousarray(Wi[rows].T).astype(bf)       # (D, 384)
        bi_c = np.ascontiguousarray(bi[rows]).astype(np.float32)  # (384,)
        in_maps.append({
            "xt": xt, "wit": wit_c, "bi_s": bi_c,
            "wot": wot, "bo_f": bo, "masks": m,
        })
    return in_maps


_CACHE = {}


def _get_runner(causal, repeat=1):
    key = (causal, repeat)
    if key not in _CACHE:
        nc = _build(causal, repeat)
        _CACHE[key] = _make_runner(nc)
    return _CACHE[key]


def kernel(x, Wi, bi, Wo, bo, causal_mask):
    causal = bool(int(np.asarray(causal_mask)))
    run = _get_runner(causal)
    in_maps = _shard_inputs(x, Wi, bi, Wo, bo, causal)
    res = run(in_maps)
    # res[c]["out_t"]: (D, RPC) fp32 = transposed rows [c*RPC, (c+1)*RPC)
    full = np.concatenate([res[c]["out_t"].T for c in range(NCORES)], axis=0)
    return np.ascontiguousarray(full.reshape(B, S, D).astype(np.float32))
